# revision 104
# baseline (speedup 1.0000x reference)
"""Trainium2 Bass kernel for nn_Attention (B=4, S=2048, D=2048, H=16, KV=4, HD=128).

Sharding (8 cores): data-parallel over batch (4) x tensor-parallel over
KV-head-group halves (2). Core c handles batch b=c//2 and q-heads
[8*(c%2), 8*(c%2)+8) == kv groups {2*(c%2), 2*(c%2)+1}. Each core produces a
partial output (its heads' contribution through wo); the host sums the two
partials per batch.

v3 (on top of the fp8-DoubleRow v2 design):
- Stage-1 PSUM split into q [128,1024] and kv [128,512] tiles: the small k/v
  chains for sbs 0-3 run during the wq DMA stream; q chains for sb0/1 are
  kt-quad-major (per quad, bank-alternating chunk pairs) so compute tracks
  the arriving weights. x streamed in s-quarters (512B contiguous runs avoid
  the <512B DMA descriptor penalty). Only one OPEN accumulation chain per
  PSUM bank (a second start=True in a bank kills the first chain's
  accumulate state).
- RoPE: ACT de-interleaves PSUM fp32 -> packed bf16 ev/od, then 6 DVE
  tensor ops per s-block run in the 4x (SBUF/bf16/packed) mode; cos/sin
  tables are bf16. Transposes batched 4-wide; their PSUM->SBUF copies run
  on DVE in 2x mode (GPSIMD cannot touch PSUM).
- Attention qsb order (1,3,2,0): qsb1 first is nearly ACT/PE balanced with
  no out-proj backlog; the deep qsbs then always run with 16 pending
  out-proj chunks as PE filler.
- exp activations batched over score-tile pairs ([128,2,512] PSUM tiles);
  causal masking via gpsimd.affine_select zeroing the probs diagonal blocks
  (replaces DVE mask adds + mask DMA). The av bank is released right after
  the AV chain by an unnormalized bf16 copy (attU); normalization happens
  in SBUF. 1/den reaches [*,512] broadcast layout via 4 thin transpose
  matmuls (x8 folded into ident8) + one gpsimd.partition_broadcast.
"""
import numpy as np

B, S, D = 4, 2048, 2048
H, KV, HD = 16, 4, 128
NREP = H // KV
SCALE = float(HD) ** -0.5
WS = 32.0                  # host weight pre-scale (power of 2)

SB = S // 128          # 16 s-blocks
KT = D // 128          # 16 contraction tiles for projections
QSB = S // 512         # 4 q-superblocks
HPC = 8                # q heads per core
GPC = 2                # kv groups per core

_compiled = {}


def _build(causal: bool):
    import concourse.bass as bass  # noqa: F401
    import concourse.tile as tile
    from concourse import bacc, mybir
    from concourse.masks import make_identity

    f32 = mybir.dt.float32
    bf16 = mybir.dt.bfloat16
    f8 = mybir.dt.float8e4
    DR = mybir.MatmulPerfMode.DoubleRow
    AF = mybir.ActivationFunctionType
    ALU = mybir.AluOpType

    nc = bacc.Bacc("TRN2")

    # x hi/lo: [0]=hi, [1]=lo.  weights hi/lo: [0]=LO, [1]=HI (two-major layout)
    xT8 = nc.dram_tensor("xT8", [2, D, S], f8, kind="ExternalInput")
    wq8 = nc.dram_tensor("wq8", [2, D, HPC * HD], f8, kind="ExternalInput")
    wkv8 = nc.dram_tensor("wkv8", [2, D, 2 * GPC * HD], f8, kind="ExternalInput")
    wo8 = nc.dram_tensor("wo8", [2, HPC * HD, D], f8, kind="ExternalInput")
    cosS = nc.dram_tensor("cosS", [128, SB, 64], bf16, kind="ExternalInput")
    sinS = nc.dram_tensor("sinS", [128, SB, 64], bf16, kind="ExternalInput")
    ones32 = nc.dram_tensor("ones32", [128, 1], bf16, kind="ExternalInput")
    outT = nc.dram_tensor("outT", [D, S], bf16, kind="ExternalOutput")

    xT4 = xT8.rearrange("two (kt p) s -> p two kt s", p=128)
    wq4 = wq8.rearrange("two (kt p) e -> p two kt e", p=128)
    wkv4 = wkv8.rearrange("two (kt p) e -> p two kt e", p=128)
    wo4 = wo8.rearrange("two (h p) d -> p two h d", p=128)

    with tile.TileContext(nc) as tc:
        with tc.tile_pool(name="persist", bufs=1) as persist:
            qTall = persist.tile([128, HPC, S], bf16, tag="qT", name="qT")
            kTall = persist.tile([128, GPC, S], bf16, tag="kT", name="kT")
            vAll = persist.tile([128, GPC, SB, 128], bf16, tag="v", name="v")
            vAll8 = persist.tile([128, GPC, SB, 128], f8, tag="v8", name="v8")
            wos = persist.tile([128, 2, HPC, D], f8, tag="wos")
            onec = persist.tile([128, 1], bf16, tag="onec")
            onec8 = persist.tile([128, 1], f8, tag="onec8")
            ident = persist.tile([128, 128], bf16, tag="ident")
            ident_f = persist.tile([128, 128], f32, tag="identf")
            make_identity(nc, ident_f)
            nc.vector.tensor_copy(out=ident, in_=ident_f)
            # 8x identity: folds the att x8 fp8 pre-scale into the
            # denominator transpose-matmul
            ident8 = persist.tile([128, 128], bf16, tag="ident8")
            nc.vector.tensor_scalar(out=ident8, in0=ident_f, scalar1=8.0,
                                    scalar2=None, op0=ALU.mult)

            bias45 = persist.tile([128, 1], f32, tag="bias45")
            nc.vector.memset(bias45, -4.5)
            cos_t = persist.tile([128, SB, 64], bf16, tag="cos")
            sin_t = persist.tile([128, SB, 64], bf16, tag="sin")

            # ------------ Stage 1: projections + RoPE + transposes ----------

            # Per s-block compute all 12 projections (8 q | 2 k | 2 v) from
            # one x load. ps columns: q 0:1024, kv 1024:1536 (K heads first).
            # 10 heads get RoPE.
            EW = (HPC + 2 * GPC) * HD  # 1536
            NR = HPC + GPC             # 10 rope heads
            with tc.tile_pool(name="w1", bufs=1) as wpool, \
                 tc.tile_pool(name="xs1", bufs=2) as xpool, \
                 tc.tile_pool(name="rs1", bufs=3) as rpool, \
                 tc.tile_pool(name="rev", bufs=4) as revp, \
                 tc.tile_pool(name="pq1", bufs=2, space="PSUM") as pqp, \
                 tc.tile_pool(name="pk1", bufs=2, space="PSUM") as pkp, \
                 tc.tile_pool(name="pt1", bufs=2, space="PSUM") as ptp:
                wtq = wpool.tile([128, 2, KT, HPC * HD], f8, tag="wtq")
                wtk = wpool.tile([128, 2, KT, 2 * GPC * HD], f8, tag="wtk")
                # x streamed in s-quarters of 512 (contiguous 512B runs).
                # DMA order: x q0 hi -> all wkv quads -> x q0 lo -> wq quads.
                # The small k/v chains for sbs 0-3 execute during the wq
                # stream; q chains for sb0/1 are kt-major so they track the
                # arriving wq quads.
                xs_q = [None] * 4
                xs_q[0] = xpool.tile([128, 2, KT, 512], f8, tag="xs",
                                     name="xs0")
                nc.sync.dma_start(out=xs_q[0][:, 0, 0:4, :],
                                  in_=xT4[:, 0, 0:4, 0:512])
                nc.sync.dma_start(out=wtk[:, 1, 0:4, :],
                                  in_=wkv4[:, 1, 0:4, :])
                nc.sync.dma_start(out=xs_q[0][:, 0, 4:16, :],
                                  in_=xT4[:, 0, 4:16, 0:512])
                nc.sync.dma_start(out=wtk[:, 1, 4:8, :],
                                  in_=wkv4[:, 1, 4:8, :])
                nc.sync.dma_start(out=xs_q[0][:, 1, :, :],
                                  in_=xT4[:, 1, :, 0:512])
                for k0 in range(8, KT, 4):
                    nc.sync.dma_start(out=wtk[:, 1, k0:k0 + 4, :],
                                      in_=wkv4[:, 1, k0:k0 + 4, :])
                for k0 in range(0, KT, 4):
                    nc.sync.dma_start(out=wtk[:, 0, k0:k0 + 4, :],
                                      in_=wkv4[:, 0, k0:k0 + 4, :])
                nc.sync.dma_start(out=cos_t, in_=cosS[:, :, :])
                nc.sync.dma_start(out=sin_t, in_=sinS[:, :, :])
                for k0 in range(0, KT, 4):
                    nc.sync.dma_start(out=wtq[:, 1, k0:k0 + 4, :],
                                      in_=wq4[:, 1, k0:k0 + 4, :])
                    nc.sync.dma_start(out=wtq[:, 0, k0:k0 + 4, :],
                                      in_=wq4[:, 0, k0:k0 + 4, :])
                nc.sync.dma_start(out=onec, in_=ones32[:, :])
                nc.vector.tensor_copy(out=onec8, in_=onec)

                def wsl_hi(kt0, kt1, n0):  # [128, 2(kt), 256] hi slice
                    if n0 < HPC * HD:
                        return wtq[:, 1, kt0:kt1, n0:n0 + 256]
                    n0 -= HPC * HD
                    return wtk[:, 1, kt0:kt1, n0:n0 + 256]

                def wsl_x(kt, n0):  # [128, 2(lo,hi), 256] cross slice
                    if n0 < HPC * HD:
                        return wtq[:, 0:2, kt, n0:n0 + 256]
                    n0 -= HPC * HD
                    return wtk[:, 0:2, kt, n0:n0 + 256]

                def emit_transposes(sb, rp):
                    # transpose 4 heads into one PSUM tile, then one batched
                    # Pool copy into the persistent [hd, s] tiles
                    for h0, cnt in ((0, 4), (4, 4), (8, 2)):
                        pt = ptp.tile([128, 1024], bf16, tag="pt")
                        for i in range(cnt):
                            nc.tensor.transpose(
                                pt[:, i * 128:(i + 1) * 128],
                                rp[:, h0 + i, :], ident)
                        pt3 = pt.rearrange("p (i c) -> p i c", c=128)
                        if h0 < HPC:
                            dst = qTall[:, h0:h0 + cnt,
                                        sb * 128:(sb + 1) * 128]
                        else:
                            dst = kTall[:, 0:GPC, sb * 128:(sb + 1) * 128]
                        # GPSIMD cannot read PSUM; DVE runs this in 2x mode
                        nc.vector.tensor_copy(out=dst, in_=pt3[:, 0:cnt, :])

                evod = {}

                def get_evod(sb):
                    if sb not in evod:
                        evb = revp.tile([128, NR, 64], bf16, tag="evb",
                                        name="evb")
                        odb = revp.tile([128, NR, 64], bf16, tag="odb",
                                        name="odb")
                        evod[sb] = (evb, odb)
                    return evod[sb]

                def kv_chain(sb):
                    xs, s0 = xs_q[sb // 4], (sb % 4) * 128
                    pskv = pkp.tile([128, 2 * GPC * HD], f32, tag="pskv",
                                    name="pskv")
                    # one open accumulation chain per PSUM bank: finish each
                    # 256-chunk (incl stop) before starting its bank-neighbor
                    for n0 in (0, 256):
                        for j in range(KT // 2):
                            nc.tensor.matmul(
                                pskv[:, n0:n0 + 256],
                                xs[:, 0, 2 * j:2 * j + 2, s0:s0 + 128],
                                wtk[:, 1, 2 * j:2 * j + 2, n0:n0 + 256],
                                start=(j == 0), stop=False, perf_mode=DR,
                                skip_group_check=True)
                        for kt in range(KT):
                            nc.tensor.matmul(
                                pskv[:, n0:n0 + 256],
                                xs[:, 0:2, kt, s0:s0 + 128],
                                wtk[:, 0:2, kt, n0:n0 + 256],
                                start=False, stop=(kt == KT - 1),
                                perf_mode=DR, skip_group_check=True)
                    kv3 = pskv.rearrange("p (h d) -> p h d", d=128)
                    evb, odb = get_evod(sb)
                    nc.scalar.copy(out=evb[:, HPC:NR, :],
                                   in_=kv3[:, 0:GPC, 0:128:2])
                    nc.scalar.copy(out=odb[:, HPC:NR, :],
                                   in_=kv3[:, 0:GPC, 1:128:2])
                    nc.scalar.copy(out=vAll[:, :, sb, :],
                                   in_=kv3[:, GPC:2 * GPC, :])
                    nc.scalar.copy(out=vAll8[:, :, sb, :],
                                   in_=kv3[:, GPC:2 * GPC, :])

                QN0 = (0, 256, 512, 768)

                def q_chain(sb, ktmajor):
                    xs, s0 = xs_q[sb // 4], (sb % 4) * 128
                    psq = pqp.tile([128, HPC * HD], f32, tag="psq",
                                   name="psq")
                    if ktmajor:
                        # per wq quad: its hi pairs then its cross tiles.
                        # Only one chunk per PSUM bank open at a time, so
                        # interleave across banks (0|512), then (256|768).
                        for chunks in ((0, 512), (256, 768)):
                            for k0 in range(0, KT, 4):
                                for n0 in chunks:
                                    for j in (k0 // 2, k0 // 2 + 1):
                                        nc.tensor.matmul(
                                            psq[:, n0:n0 + 256],
                                            xs[:, 0, 2 * j:2 * j + 2,
                                               s0:s0 + 128],
                                            wtq[:, 1, 2 * j:2 * j + 2,
                                                n0:n0 + 256],
                                            start=(j == 0), stop=False,
                                            perf_mode=DR,
                                            skip_group_check=True)
                                for n0 in chunks:
                                    for kt in range(k0, k0 + 4):
                                        nc.tensor.matmul(
                                            psq[:, n0:n0 + 256],
                                            xs[:, 0:2, kt, s0:s0 + 128],
                                            wtq[:, 0:2, kt, n0:n0 + 256],
                                            start=False,
                                            stop=(kt == KT - 1),
                                            perf_mode=DR,
                                            skip_group_check=True)
                    else:
                        for n0 in QN0:
                            for j in range(KT // 2):
                                nc.tensor.matmul(
                                    psq[:, n0:n0 + 256],
                                    xs[:, 0, 2 * j:2 * j + 2, s0:s0 + 128],
                                    wtq[:, 1, 2 * j:2 * j + 2, n0:n0 + 256],
                                    start=(j == 0), stop=False, perf_mode=DR,
                                    skip_group_check=True)
                            for kt in range(KT):
                                nc.tensor.matmul(
                                    psq[:, n0:n0 + 256],
                                    xs[:, 0:2, kt, s0:s0 + 128],
                                    wtq[:, 0:2, kt, n0:n0 + 256],
                                    start=False, stop=(kt == KT - 1),
                                    perf_mode=DR, skip_group_check=True)
                    q3 = psq.rearrange("p (h d) -> p h d", d=128)
                    evb, odb = get_evod(sb)
                    nc.scalar.copy(out=evb[:, 0:HPC, :],
                                   in_=q3[:, :, 0:128:2])
                    nc.scalar.copy(out=odb[:, 0:HPC, :],
                                   in_=q3[:, :, 1:128:2])

                def rope_tt(sb):
                    evb, odb = evod.pop(sb)
                    rp = rpool.tile([128, NR, 128], bf16, tag="rope")
                    t1 = rpool.tile([128, NR, 64], bf16, tag="t1")
                    t2 = rpool.tile([128, NR, 64], bf16, tag="t2")
                    cb = cos_t[:, None, sb, :].broadcast_to([128, NR, 64])
                    sn = sin_t[:, None, sb, :].broadcast_to([128, NR, 64])
                    nc.vector.tensor_tensor(out=t1, in0=evb, in1=cb,
                                            op=ALU.mult)
                    nc.vector.tensor_tensor(out=t2, in0=odb, in1=sn,
                                            op=ALU.mult)
                    nc.vector.tensor_tensor(out=rp[:, :, 0:64], in0=t1,
                                            in1=t2, op=ALU.subtract)
                    nc.vector.tensor_tensor(out=t1, in0=evb, in1=sn,
                                            op=ALU.mult)
                    nc.vector.tensor_tensor(out=t2, in0=odb, in1=cb,
                                            op=ALU.mult)
                    nc.vector.tensor_tensor(out=rp[:, :, 64:128], in0=t1,
                                            in1=t2, op=ALU.add)
                    return rp

                # k/v projections for the first quarter run during the wq
                # stream; q chains for sb0/1 track the arriving wq quads
                for sb in range(4):
                    kv_chain(sb)
                q_chain(0, True)
                q_chain(1, True)

                prev = None  # (sb, rp) whose transposes are deferred one iter
                for sb in range(SB):
                    q, qi = sb // 4, sb % 4
                    if qi == 0 and q < 3:
                        xs_q[q + 1] = xpool.tile([128, 2, KT, 512], f8,
                                                 tag="xs", name="xsn")
                        for v in range(2):
                            nc.sync.dma_start(
                                out=xs_q[q + 1][:, v, :, :],
                                in_=xT4[:, v, :, (q + 1) * 512:(q + 2) * 512])
                    if sb in (3, 6, 9, 12):
                        c = {3: 0, 6: 1, 9: 2, 12: 3}[sb]
                        for v in range(2):
                            nc.sync.dma_start(
                                out=wos[:, v, :, 512 * c:512 * (c + 1)],
                                in_=wo4[:, v, :, 512 * c:512 * (c + 1)])
                    if sb >= 4:
                        kv_chain(sb)
                    if sb >= 2:
                        q_chain(sb, False)
                    rp = rope_tt(sb)
                    if prev is not None:
                        emit_transposes(*prev)
                    prev = (sb, rp)
                emit_transposes(*prev)

            # ------------ Stage 2+3: attention + out-projection -------------
            with tc.tile_pool(name="pr2", bufs=4) as prpool, \
                 tc.tile_pool(name="pr0", bufs=2) as pr0pool, \
                 tc.tile_pool(name="att2", bufs=2) as attpool, \
                 tc.tile_pool(name="dn2", bufs=3) as dnpool, \
                 tc.tile_pool(name="o2", bufs=4) as opool, \
                 tc.tile_pool(name="psc", bufs=2, space="PSUM") as pscp, \
                 tc.tile_pool(name="pav", bufs=2, space="PSUM") as pavp, \
                 tc.tile_pool(name="pou", bufs=2, space="PSUM") as poup:

                pending = []  # (qsb, att_tile, m) out-proj chunks not yet run

                def oproj_chunk(use_psc=False, drain=False):
                    if not pending:
                        return
                    oq, oatt, m = pending.pop(0)
                    if use_psc:
                        pot = pscp.tile([128, 2, 512], f32, tag="sc",
                                        name="podrain")
                        po = pot[:, 0, :]
                    else:
                        po = poup.tile([128, 512], f32, tag="po")
                    ms = slice(m * 128, (m + 1) * 128)
                    for j in range(HPC // 2):
                        e = 2 * j
                        nc.tensor.matmul(
                            po[:, 0:512],
                            wos[:, 1, e:e + 2, ms],
                            oatt[:, e:e + 2, 0, 0:512],
                            start=(j == 0), stop=False, perf_mode=DR,
                            skip_group_check=True)
                    for e in range(HPC):
                        nc.tensor.matmul(
                            po[:, 0:512],
                            wos[:, 0:2, e, ms],
                            oatt[:, e, 0:2, 0:512],
                            start=False, stop=(e == HPC - 1),
                            perf_mode=DR, skip_group_check=True)
                    ot = opool.tile([128, 512], bf16, tag="ot")
                    nc.vector.tensor_copy(out=ot, in_=po[:, 0:512])
                    nc.sync.dma_start(
                        out=outT[m * 128:(m + 1) * 128,
                                 oq * 512:(oq + 1) * 512],
                        in_=ot)

                for qsb in (1, 2, 3, 0) if causal else (0, 1, 2, 3):
                    att = attpool.tile([128, HPC, 512], bf16, tag="att")
                    att8 = attpool.tile([128, HPC, 2, 512], f8, tag="att8")
                    maxkt = (qsb + 1) * 4 if causal else SB
                    q0g = qsb * 512
                    for g in range(GPC):
                        for r in range(NREP):
                            h = g * NREP + r
                            # qsb0 (short causal rows) keeps bf16 probs;
                            # deeper qsbs use fp8e4 probs with a -4.5 logit
                            # bias (range fit; softmax normalization cancels
                            # the scale and most of the quantization)
                            lo8 = causal and qsb not in (0, 1)
                            if lo8:
                                probs = prpool.tile([128, SB, 512], f8,
                                                    tag="probs8",
                                                    name="probs8")
                            else:
                                probs = pr0pool.tile([128, SB, 512], bf16,
                                                     tag="probs",
                                                     name="probs")
                            for tp in range(maxkt // 2):
                                t0 = 2 * tp
                                # pair cover width (even tile's causal edge)
                                ql0 = (max(0, t0 * 128 - q0g)
                                       if causal else 0)
                                diag = causal and t0 * 128 >= q0g
                                sc = pscp.tile([128, 2, 512], f32, tag="sc")
                                if lo8 and diag:
                                    # fp8 diag pair: zero the odd tile's
                                    # junk prefix early (off the exp->AV
                                    # path), exp each tile at its true edge,
                                    # then narrow 128-wide selects
                                    nc.gpsimd.memset(
                                        probs[:, t0 + 1, ql0:ql0 + 128], 0.0)
                                    for tl in range(2):
                                        t = t0 + tl
                                        qlt = t * 128 - q0g
                                        nc.tensor.matmul(
                                            sc[:, tl, qlt:512],
                                            kTall[:, g,
                                                  t * 128:(t + 1) * 128],
                                            qTall[:, h,
                                                  q0g + qlt:q0g + 512],
                                            start=True, stop=True)
                                        nc.scalar.activation(
                                            out=probs[:, t, qlt:512],
                                            in_=sc[:, tl, qlt:512],
                                            func=AF.Exp, scale=SCALE,
                                            bias=bias45)
                                        pd = probs[:, t, qlt:qlt + 128]
                                        nc.gpsimd.affine_select(
                                            out=pd, in_=pd,
                                            pattern=[[1, 128]],
                                            compare_op=ALU.is_ge,
                                            fill=0.0, base=0,
                                            channel_multiplier=-1)
                                    continue
                                for tl in range(2):
                                    t = t0 + tl
                                    nc.tensor.matmul(
                                        sc[:, tl, ql0:512],
                                        kTall[:, g, t * 128:(t + 1) * 128],
                                        qTall[:, h, q0g + ql0:q0g + 512],
                                        start=True, stop=True)
                                nc.scalar.activation(
                                    out=probs[:, t0:t0 + 2, ql0:512],
                                    in_=sc[:, :, ql0:512], func=AF.Exp,
                                    scale=SCALE,
                                    bias=bias45 if lo8 else 0.0)
                                if diag:
                                    for tl in range(2):
                                        t = t0 + tl
                                        qlt = t * 128 - q0g
                                        pd = probs[:, t, qlt:qlt + 128]
                                        nc.gpsimd.affine_select(
                                            out=pd, in_=pd,
                                            pattern=[[1, 128]],
                                            compare_op=ALU.is_ge,
                                            fill=0.0, base=0,
                                            channel_multiplier=-1)
                            if not (qsb in (3, 2, 0) and g == 0 and r <= 1):
                                oproj_chunk()
                            # AV accumulation (x WS via v scaling); fp8 path
                            # runs DoubleRow over k-tile pairs (half cost)
                            av = pavp.tile([128, 512], f32, tag="av")
                            if lo8:
                                npair = maxkt // 2
                                for tp in range(npair):
                                    t0 = 2 * tp
                                    ql0 = max(0, t0 * 128 - q0g)
                                    nc.tensor.matmul(
                                        av[:, ql0:512],
                                        vAll8[:, g, t0:t0 + 2, :],
                                        probs[:, t0:t0 + 2, ql0:512],
                                        start=(tp == 0),
                                        stop=(tp == npair - 1),
                                        perf_mode=DR,
                                        skip_group_check=True)
                            else:
                                for t in range(maxkt):
                                    ql = (max(0, t * 128 - q0g)
                                          if causal else 0)
                                    nc.tensor.matmul(
                                        av[:, ql:512], vAll[:, g, t, :],
                                        probs[:, t, ql:512],
                                        start=(t == 0),
                                        stop=(t == maxkt - 1),
                                        skip_group_check=True)
                            # release the av bank early: copy the
                            # unnormalized sum to SBUF, normalize there
                            attU = dnpool.tile([128, 512], bf16, tag="attU")
                            if qsb == 0:
                                nc.scalar.copy(out=attU, in_=av)
                            else:
                                nc.vector.tensor_copy(out=attU, in_=av)
                            # denominators: probs-stationary chains into the
                            # rr1 tile (cols 508:512; shares the av ring --
                            # attU above must read av first)
                            rr1 = pavp.tile([128, 512], f32, tag="av",
                                            name="rr1")
                            onex = onec8 if lo8 else onec
                            for m in range(4):
                                tmax = (min(maxkt, 4 * qsb + m + 1)
                                        if causal else SB)
                                for t in range(tmax):
                                    nc.tensor.matmul(
                                        rr1[:, 508 + m:509 + m],
                                        probs[:, t, m * 128:(m + 1) * 128],
                                        onex,
                                        start=(t == 0), stop=(t == tmax - 1),
                                        skip_group_check=True)
                            rrs = dnpool.tile([128, 4], bf16, tag="rrs")
                            with nc.allow_low_precision(reason="softmax recip"):
                                nc.vector.reciprocal(
                                    out=rrs, in_=rr1[:, 508:512])
                            # gather 8/den onto partition 0 as a [1, 512]
                            # row (4 thin transpose-matmuls, x8 via ident8;
                            # clobbers den@p0 only after the recip read),
                            # then one Pool partition_broadcast
                            for m in range(4):
                                nc.tensor.matmul(
                                    rr1[0:1, m * 128:(m + 1) * 128],
                                    rrs[:, m:m + 1], ident8,
                                    start=True, stop=True)
                            rrc = dnpool.tile([1, 512], bf16, tag="rrc")
                            nc.vector.tensor_copy(out=rrc, in_=rr1[0:1, :])
                            rsbs = dnpool.tile([128, 512], bf16, tag="rsbs")
                            nc.gpsimd.partition_broadcast(rsbs, rrc)
                            oproj_chunk()
                            # fused normalize: att = attU * (1/den32)
                            nc.vector.tensor_tensor(
                                out=att[:, h, :], in0=attU, in1=rsbs,
                                op=ALU.mult)
                            if qsb == 0:
                                nc.scalar.copy(out=att8[:, h, 0, :],
                                               in_=att[:, h, :])
                            else:
                                nc.vector.tensor_copy(out=att8[:, h, 0, :],
                                                      in_=att[:, h, :])
                            nc.vector.tensor_tensor(
                                out=att8[:, h, 1, :], in0=att[:, h, :],
                                in1=att8[:, h, 0, :], op=ALU.subtract)
                    pending.extend((qsb, att8, m) for m in range(KT))
                # drain tail, alternating PSUM banks to double-buffer
                i = 0
                while pending:
                    oproj_chunk(use_psc=(i % 2 == 1), drain=True)
                    i += 1

    nc.compile()
    return nc


def _get_nc(causal: bool):
    if causal not in _compiled:
        _compiled[causal] = _build(causal)
    return _compiled[causal]


def _split8(a):
    import ml_dtypes
    E4 = ml_dtypes.float8_e4m3
    hi = a.astype(E4)
    lo = (a - hi.astype(np.float32)).astype(E4)
    return hi, lo


def kernel(x, freqs_cis, mask, wq, wk, wv, wo):
    import ml_dtypes
    from concourse.bass_utils import run_bass_kernel_spmd
    BF = ml_dtypes.bfloat16

    x = np.asarray(x, dtype=np.float32)
    freqs_cis = np.asarray(freqs_cis, dtype=np.float32)
    mask = np.asarray(mask, dtype=np.float32)
    wq = np.asarray(wq, dtype=np.float32)
    wk = np.asarray(wk, dtype=np.float32)
    wv = np.asarray(wv, dtype=np.float32)
    wo = np.asarray(wo, dtype=np.float32)

    tri = np.tril(np.ones((S, S), dtype=bool))
    causal = bool((mask[tri] == 0.0).all() and (mask[~tri] < -1e30).all())
    if not causal and not (mask == 0.0).all():
        return _numpy_ref(x, freqs_cis, mask, wq, wk, wv, wo)

    nc = _get_nc(causal)

    cos = freqs_cis[:, :, 0] / WS
    sin = freqs_cis[:, :, 1] / WS
    cosS = np.ascontiguousarray(
        cos.reshape(SB, 128, 64).transpose(1, 0, 2)).astype(BF)
    sinS = np.ascontiguousarray(
        sin.reshape(SB, 128, 64).transpose(1, 0, 2)).astype(BF)
    ones32 = np.full((128, 1), WS, dtype=BF)

    def pack2(a, b):  # [D, E], [D, E] -> [2, D, E]
        return np.ascontiguousarray(np.stack([a, b], axis=0))

    in_maps = []
    for c in range(8):
        b, i = c // 2, c % 2
        xh, xl = _split8(x[b].T)
        wqh, wql = _split8(wq[1024 * i:1024 * (i + 1), :].T * WS)
        wkvf = np.concatenate(
            [wk[256 * i:256 * (i + 1), :].T,
             wv[256 * i:256 * (i + 1), :].T], axis=1) * WS
        wkh, wkl = _split8(wkvf)
        in_maps.append({
            "xT8": pack2(xh, xl),
            "wq8": pack2(wql, wqh),    # weights: [:,0,:]=lo, [:,1,:]=hi
            "wkv8": pack2(wkl, wkh),
            "wo8": pack2(*reversed(_split8(
                np.ascontiguousarray(
                    wo[:, 1024 * i:1024 * (i + 1)].T) * WS))),
            "cosS": cosS, "sinS": sinS,
            "ones32": ones32,
        })

    res = run_bass_kernel_spmd(nc, in_maps, core_ids=list(range(8)))
    out = np.empty((B, S, D), dtype=np.float32)
    for b in range(B):
        out[b] = (res.results[2 * b]["outT"].T.astype(np.float32)
                  + res.results[2 * b + 1]["outT"].T.astype(np.float32)) \
            * (1.0 / (WS * 8.0))
    return out


def _numpy_ref(x, freqs_cis, mask, wq, wk, wv, wo):
    xq = (x @ wq.T).reshape(B, S, H, HD)
    xk = (x @ wk.T).reshape(B, S, KV, HD)
    xv = (x @ wv.T).reshape(B, S, KV, HD)

    def rope(xh):
        x2 = xh.reshape(*xh.shape[:-1], HD // 2, 2)
        fc = freqs_cis[None, :, None, :, :]
        real = x2[..., 0] * fc[..., 0] - x2[..., 1] * fc[..., 1]
        imag = x2[..., 0] * fc[..., 1] + x2[..., 1] * fc[..., 0]
        return np.concatenate([real, imag], axis=-1)

    xq, xk = rope(xq), rope(xk)
    q = xq.reshape(B, S, KV, NREP, HD)
    sc = np.einsum('bqgrd,bkgd->bgrqk', q, xk) * SCALE + mask[None, None, None]
    sc = sc - sc.max(axis=-1, keepdims=True)
    p = np.exp(sc)
    p /= p.sum(axis=-1, keepdims=True)
    o = np.einsum('bgrqk,bkgd->bqgrd', p, xv).reshape(B, S, H * HD)
    return (o @ wo.T).astype(np.float32)
